# revision 25
# baseline (speedup 1.0000x reference)
"""Trainium2 Bass kernel for nn_BCAModule (bilateral cross-attention).

Full inputs in, full outputs out. Internally sharded over 8 NeuronCores:
core c handles batch b = c // 4 and query rows 32*(c%4) .. 32*(c%4)+32
(N_loc = 4096 of the N = 16384 queries). Pooled K/V ([64, 1024]) is built
cooperatively: each core pools its own spatial quarter ([64, 256]) and the
4-core group all-gathers.

All heavy tensors ride in fp16 (inputs cast host-side, matmuls fp16 with
f32 PSUM accumulation, output written fp16 and upcast host-side). sim is
in [-9, 8] for these inputs so exp(sim) fits fp16 range comfortably.

Every hot matmul runs with a full K=128 contraction: the PE clock gate
(HAM) only releases the 2.4 GHz clock when array utilization is high, and
half-height (K=64) matmuls run at 1.2 GHz forever. So:
  - the input projection is padded host-side to 768 channels (6x K=128),
  - the y projection packs two n-tiles via a block-diagonal [[Ay],[Ay]],
  - sim packs two 64-wide m chunks via block-diagonal fy tiles against a
    duplicated-fx rhs,
  - the up-projection packs two 60-channel output chunks via a
    block-diagonal wu (built host-side) against a duplicated-fout rhs.

Pipelining: pools are hoisted out of the repeat loop (per-tag buffer
rotation); the repeat loop is software-pipelined so each n-tile
interleaves attention of iteration k-1 with projections of iteration k,
keeping the PE stream dense. Loads issue on the sync(SP) HWDGE ring,
stores on the scalar(ACT) ring, gather-unpack on gpsimd SWDGE.
"""

import numpy as np

B, CX, CM, H, W = 2, 720, 64, 128, 128
CXP = 768             # padded input channels (6 x 128)
NCORES = 8
RB = 32               # image rows per core
NL = RB * W           # 4096 local queries
ML = (RB // 4) * (W // 4)   # 256 local pooled positions
M = 4 * ML            # 1024 pooled positions per batch
KC = CXP // 128       # 6 proj contraction chunks of 128
KPO = 120             # output-channel chunk (two 60-halves per packed MM)
OC = CX // KPO        # 6
NT = 512              # n tile
NTN = NL // NT        # 8
MCH = 128             # m chunk
NMC = M // MCH        # 8
GR = 2 * ML           # gather rows per core: fs^T block + fy block

_CACHE = {}


def _build_nc(repeat=1, phases=(1, 2)):
    import concourse.bass as bass
    from concourse import bacc
    import concourse.mybir as mybir
    import concourse.tile as tile
    from concourse.masks import make_identity

    F32 = mybir.dt.float32
    F16 = mybir.dt.float16
    AF = mybir.ActivationFunctionType
    ALU = mybir.AluOpType

    nc = bacc.Bacc(None)

    xq_d = nc.dram_tensor("xq", [CXP, NL], F16, kind="ExternalInput")
    yq_d = nc.dram_tensor("yq", [CM, NL], F16, kind="ExternalInput")
    wks_d = nc.dram_tensor("wks", [CXP, 128], F16, kind="ExternalInput")
    wyd_d = nc.dram_tensor("wyd", [128, 128], F16, kind="ExternalInput")
    wud_d = nc.dram_tensor("wud", [128, CXP], F16, kind="ExternalInput")
    bxs_d = nc.dram_tensor("bxs", [128, 1], F32, kind="ExternalInput")
    by_d = nc.dram_tensor("by", [CM, 1], F32, kind="ExternalInput")
    bu_d = nc.dram_tensor("bu", [CXP, 1], F32, kind="ExternalInput")
    out_d = nc.dram_tensor("out", [CXP, NL], F16, kind="ExternalOutput")

    with tile.TileContext(nc) as tc:
        with (
            tc.tile_pool(name="wpool", bufs=1) as wp,
            tc.tile_pool(name="xpool", bufs=1) as xp,
            tc.tile_pool(name="persist", bufs=1) as pers,
            tc.tile_pool(name="p1sb", bufs=1) as p1,
            tc.tile_pool(name="p2sb", bufs=1) as p2,
            tc.tile_pool(name="pjps", bufs=1, space="PSUM") as pj,
            tc.tile_pool(name="p2ps", bufs=1, space="PSUM") as p2p,
            tc.tile_pool(name="dram", bufs=2, space="DRAM") as dp,
        ):
            # ---------------- weights / constants (once) ----------------
            w_ks_t = wp.tile([128, KC * 128], F16, tag="wks")
            nc.sync.dma_start(
                w_ks_t[:].rearrange("p (k m) -> p k m", k=KC),
                wks_d[:].rearrange("(k p) m -> p k m", k=KC),
            )
            w_ks = w_ks_t[:]

            w_yd_t = wp.tile([128, 128], F16, tag="wyd")
            nc.sync.dma_start(w_yd_t[:], wyd_d[:])
            w_yd = w_yd_t[:]

            w_ud_t = wp.tile([128, CXP], F16, tag="wud")
            nc.sync.dma_start(w_ud_t[:], wud_d[:])
            w_ud = w_ud_t[:]

            bxs_sb = wp.tile([128, 1], F32, tag="bxs")
            nc.sync.dma_start(bxs_sb[:], bxs_d[:])
            by_sb = wp.tile([CM, 1], F32, tag="by")
            nc.sync.dma_start(by_sb[:], by_d[:])
            bu_sb = wp.tile([128, KC], F32, tag="bu")
            nc.sync.dma_start(
                bu_sb[:].rearrange("p (k o) -> p k o", k=KC),
                bu_d[:].rearrange("(k p) o -> p k o", k=KC),
            )

            ident = wp.tile([128, 128], F16, tag="ident")
            make_identity(nc, ident[:])
            ones_f = wp.tile([128, 1], F16, tag="ones_f")
            nc.gpsimd.memset(ones_f[:], 1.0)
            ones_row = wp.tile([1, CM], F32, tag="ones_row")
            nc.gpsimd.memset(ones_row[:], 1.0)


            def phase1_head(it):
                st = {}
                st["g_in"] = dp.tile([GR, CM], F16, tag="g_in", name="g_in")
                st["g_out"] = dp.tile([4 * GR, CM], F16, tag="g_out",
                                      name="g_out")
                st["fxd"] = pers.tile([128, NL], F16, tag="fxd", bufs=2,
                                      name="fxd")
                # block-diagonal fy tiles [[fy_a, 0], [0, fy_b]],
                # double-buffered across reps; re-zeroed each rep
                st["fyd"] = [
                    pers.tile([128, 128], F16, tag=f"fyd{mc}", bufs=2,
                              name=f"fyd{mc}")
                    for mc in range(NMC)
                ]
                for mc in range(NMC):
                    nc.gpsimd.memset(st["fyd"][mc][:], 0.0)
                st["fsa"] = [
                    pers.tile([MCH, 65], F16, tag=f"fsa{mc}", bufs=2,
                              name=f"fsa{mc}")
                    for mc in range(NMC)
                ]
                st["x"] = [
                    xp.tile([128, NL], F16, tag=f"x{k}", bufs=2, name=f"x{k}")
                    for k in range(KC)
                ]
                for k in range(KC):
                    nc.sync.dma_start(
                        st["x"][k][:], xq_d[k * 128:(k + 1) * 128, :])
                # y in packed layout: rows 0:64 = first half cols,
                # rows 64:128 = second half cols
                st["y2"] = p1.tile([128, NL // 2], F16, tag="y2", bufs=2,
                                   name="y2")
                nc.sync.dma_start(st["y2"][0:CM, :], yq_d[:, 0:NL // 2])
                nc.sync.dma_start(st["y2"][CM:128, :], yq_d[:, NL // 2:NL])
                st["fy_p1"] = p1.tile([CM, RB * 32], F32, tag="pool_p1",
                                      bufs=2, name="fy_p1")
                st["fs_p1"] = p1.tile([CM, RB * 32], F16, tag="fs_p1", bufs=2,
                                      name="fs_p1")
                return st

            def proj_nt(st, nt):
                ns = slice(nt * NT, (nt + 1) * NT)
                if nt < NTN // 2:
                    # packed y projection: one K=128 MM covers n-tiles
                    # nt and nt + 4
                    ps = pj.tile([128, NT], F32, tag="pp", bufs=2, name="ps_y")
                    nc.tensor.matmul(
                        ps[:], w_yd, st["y2"][:, ns], start=True, stop=True)
                    nc.vector.tensor_reduce(
                        st["fy_p1"][:, nt * 128:(nt + 1) * 128],
                        ps[0:CM, :].rearrange("p (a w) -> p a w", w=4),
                        axis=mybir.AxisListType.X, op=ALU.max,
                    )
                    nc.vector.tensor_reduce(
                        st["fy_p1"][:, (nt + 4) * 128:(nt + 5) * 128],
                        ps[CM:128, :].rearrange("p (a w) -> p a w", w=4),
                        axis=mybir.AxisListType.X, op=ALU.max,
                    )
                pp = pj.tile([128, NT], F32, tag="pp", bufs=2, name="pp")
                for k in range(KC):
                    nc.tensor.matmul(
                        pp[:],
                        w_ks[:, k * 128:(k + 1) * 128],
                        st["x"][k][:, ns],
                        start=(k == 0),
                        stop=(k == KC - 1),
                    )
                # fx duplicated to both partition halves (sim rhs);
                # fself pooled straight off PSUM (bias folded post-pool)
                nc.vector.tensor_scalar_add(
                    st["fxd"][0:CM, ns], pp[0:CM, :], bxs_sb[0:CM, :])
                nc.vector.tensor_scalar_add(
                    st["fxd"][CM:128, ns], pp[0:CM, :], bxs_sb[0:CM, :])
                nc.vector.tensor_reduce(
                    st["fs_p1"][:, nt * 128:(nt + 1) * 128],
                    pp[CM:128, :].rearrange("p (a w) -> p a w", w=4),
                    axis=mybir.AxisListType.X, op=ALU.max,
                )

            def phase1_tail(st):
                fy_pool = p1.tile([CM, ML], F32, tag="fy_pool", bufs=2)
                nc.vector.tensor_reduce(
                    fy_pool[:],
                    st["fy_p1"][:].rearrange(
                        "p (hb hh wb) -> p hb wb hh", hb=RB // 4, hh=4),
                    axis=mybir.AxisListType.X, op=ALU.max,
                )
                fy_poolb = p1.tile([CM, ML], F16, tag="fy_poolb", bufs=2)
                nc.vector.tensor_scalar_add(fy_poolb[:], fy_pool[:], by_sb[:])
                fs_pool = p1.tile([CM, ML], F16, tag="fs_pool", bufs=2)
                nc.vector.tensor_reduce(
                    fs_pool[:],
                    st["fs_p1"][:].rearrange(
                        "p (hb hh wb) -> p hb wb hh", hb=RB // 4, hh=4),
                    axis=mybir.AxisListType.X, op=ALU.max,
                )
                fs_poolb = p1.tile([CM, ML], F16, tag="fs_poolb", bufs=2)
                nc.vector.tensor_scalar_add(
                    fs_poolb[:], fs_pool[:], bxs_sb[CM:128, :])
                # gather block: rows 0:256 = fs^T, rows 256:512 = fy
                fst = p1.tile([128, 128], F16, tag="fst", bufs=2)
                for j in range(2):
                    tps = p2p.tile([128, CM], F16, tag="up", bufs=2, name="tps")
                    nc.tensor.transpose(
                        tps[:], fs_poolb[:, j * 128:(j + 1) * 128],
                        ident[0:CM, 0:CM],
                    )
                    nc.vector.tensor_copy(fst[:, j * CM:(j + 1) * CM], tps[:])
                nc.scalar.dma_start(
                    st["g_in"][0:ML, :].rearrange("(j p) c -> p j c", p=128),
                    fst[:].rearrange("p (j c) -> p j c", j=2),
                )
                nc.scalar.dma_start(
                    st["g_in"][ML:GR, :].rearrange("(c a) b -> c (a b)", a=4),
                    fy_poolb[:],
                )
                if "nocc" not in phases:
                    nc.gpsimd.collective_compute(
                        "AllGather",
                        ALU.bypass,
                        replica_groups=[[0, 1, 2, 3], [4, 5, 6, 7]],
                        ins=[st["g_in"][:].opt()],
                        outs=[st["g_out"][:].opt()],
                    )
                # unpack gathered K/V: fsa direct, fy into block-diag tiles
                for r in range(4):
                    base = r * GR
                    for h in range(2):
                        mc = 2 * r + h
                        nc.gpsimd.dma_start(
                            st["fsa"][mc][:, 0:CM],
                            st["g_out"][base + h * 128:base + (h + 1) * 128, :],
                        )
                        nc.vector.tensor_copy(
                            st["fsa"][mc][:, CM:65], ones_f[:])
                    fy_rows = st["g_out"][base + ML:base + GR, :].rearrange(
                        "(c a) b -> a c b", a=4)
                    for a in range(4):
                        mc = 2 * r + a // 2
                        h = a % 2
                        nc.gpsimd.dma_start(
                            st["fyd"][mc][h * CM:(h + 1) * CM,
                                          h * CM:(h + 1) * CM],
                            fy_rows[a],
                        )

            def attn_sim(st, nt):
                ns = slice(nt * NT, (nt + 1) * NT)
                et = p2.tile([128, NMC * NT], F16, tag="et", bufs=2)
                for mc in range(NMC):
                    sim = p2p.tile([128, NT], F32, tag="sim", bufs=3,
                                   name="sim")
                    nc.tensor.matmul(
                        sim[:], st["fyd"][mc][:], st["fxd"][:, ns],
                        start=True, stop=True,
                    )
                    nc.scalar.activation(
                        et[:, mc * NT:(mc + 1) * NT], sim[:], AF.Exp,
                    )
                return et

            def attn_tail(st, nt, et):
                ns = slice(nt * NT, (nt + 1) * NT)
                fo = p2p.tile([65, NT], F32, tag="fo", bufs=1)
                for mc in range(NMC):
                    nc.tensor.matmul(
                        fo[:], st["fsa"][mc][:], et[:, mc * NT:(mc + 1) * NT],
                        start=(mc == 0), stop=(mc == NMC - 1),
                    )
                zrow = p2.tile([1, NT], F32, tag="zrow", bufs=2)
                nc.vector.tensor_copy(zrow[:], fo[CM:65, :])
                rz = p2.tile([1, NT], F32, tag="rz", bufs=2)
                nc.vector.reciprocal_approx_fast(rz[:], zrow[:])
                rzb_ps = p2p.tile([128, NT], F32, tag="sim", bufs=3, name="rzb")
                nc.tensor.matmul(
                    rzb_ps[0:CM, :], ones_row[:], rz[:], start=True, stop=True)
                rzb_sb = p2.tile([CM, NT], F32, tag="rzb_sb", bufs=2)
                nc.vector.tensor_copy(rzb_sb[:], rzb_ps[0:CM, :])
                # fout duplicated to both halves (packed up-proj rhs)
                fout2 = p2.tile([128, NT], F16, tag="fout2", bufs=2)
                nc.vector.tensor_tensor(
                    fout2[0:CM, :], fo[0:CM, :], rzb_sb[:], op=ALU.mult)
                nc.vector.tensor_tensor(
                    fout2[CM:128, :], fo[0:CM, :], rzb_sb[:], op=ALU.mult)
                ob = p2.tile([128, KC * NT], F16, tag="ob", bufs=2)
                for ot in range(KC):
                    xr = p2.tile([128, NT], F16, tag="xr", bufs=6)
                    nc.sync.dma_start(xr[:], xq_d[ot * 128:(ot + 1) * 128, ns])
                    up = p2p.tile([128, NT], F32, tag="up", bufs=2, name="up")
                    nc.tensor.matmul(
                        up[:], w_ud[:, ot * 128:(ot + 1) * 128],
                        fout2[:], start=True, stop=True,
                    )
                    nc.vector.scalar_tensor_tensor(
                        ob[:, ot * NT:(ot + 1) * NT], up[:],
                        bu_sb[:, ot:ot + 1], xr[:],
                        op0=ALU.add, op1=ALU.add,
                    )
                nc.scalar.dma_start(
                    out_d[:, ns].rearrange("(o p) n -> p o n", o=KC),
                    ob[:].rearrange("p (o n) -> p o n", o=KC),
                )

            # software pipeline: iteration `it` runs phase 1 of rep `it`
            # interleaved (per n-tile) with the attention of rep `it-1`,
            # so projection matmuls fill the PE bubbles left by the
            # exp/normalize chains and keep the HAM clock warm.
            prev = None
            for it in range(repeat + 1):
                cur = phase1_head(it) if it < repeat else None
                for nt in range(NTN):
                    et = attn_sim(prev, nt) if prev is not None else None
                    if cur is not None:
                        proj_nt(cur, nt)
                    if prev is not None:
                        attn_tail(prev, nt, et)
                if cur is not None:
                    phase1_tail(cur)
                prev = cur

    nc.finalize()
    return nc


def _fold(W1, s1, b1, W2, s2, b2):
    W1 = W1.astype(np.float64)
    W2 = W2.astype(np.float64)
    A1 = s1.astype(np.float64)[:, None] * W1
    A2 = s2.astype(np.float64)[:, None] * W2
    A = A2 @ A1
    c = A2 @ b1.astype(np.float64) + b2.astype(np.float64)
    return A, c


def _get_runner():
    if "runner" in _CACHE:
        return _CACHE["runner"]

    import jax
    import concourse.mybir as mybir
    from jax.sharding import Mesh, PartitionSpec
    from jax.experimental.shard_map import shard_map
    from concourse.bass2jax import (
        _bass_exec_p, install_neuronx_cc_hook, partition_id_tensor,
    )

    nc = _build_nc()
    install_neuronx_cc_hook()

    partition_name = nc.partition_id_tensor.name if nc.partition_id_tensor else None
    in_names, out_names, out_avals, zero_shapes = [], [], [], []
    for alloc in nc.m.functions[0].allocations:
        if not isinstance(alloc, mybir.MemoryLocationSet):
            continue
        if getattr(alloc, "kind", None) == "ExternalInput":
            name = alloc.memorylocations[0].name
            if name != partition_name:
                in_names.append(name)
        elif getattr(alloc, "kind", None) == "ExternalOutput":
            name = alloc.memorylocations[0].name
            out_names.append(name)
            shape = tuple(alloc.tensor_shape)
            dtype = mybir.dt.np(alloc.dtype)
            out_avals.append(jax.core.ShapedArray(shape, dtype))
            zero_shapes.append((shape, dtype))

    n_params = len(in_names)
    n_outs = len(out_avals)
    all_in_names = list(in_names) + list(out_names)
    if partition_name is not None:
        all_in_names.append(partition_name)

    def _body(*args):
        operands = list(args)
        if partition_name is not None:
            operands.append(partition_id_tensor())
        outs = _bass_exec_p.bind(
            *operands,
            out_avals=tuple(out_avals),
            in_names=tuple(all_in_names),
            out_names=tuple(out_names),
            lowering_input_output_aliases=(),
            sim_require_finite=True,
            sim_require_nnan=True,
            nc=nc,
        )
        return tuple(outs)

    devices = jax.devices()[:NCORES]
    mesh = Mesh(np.asarray(devices), ("core",))
    in_specs = (PartitionSpec("core"),) * (n_params + n_outs)
    out_specs = (PartitionSpec("core"),) * n_outs
    sharded = jax.jit(
        shard_map(_body, mesh=mesh, in_specs=in_specs, out_specs=out_specs,
                  check_rep=False),
        keep_unused=True,
    )

    runner = {
        "sharded": sharded,
        "in_names": in_names,
        "out_names": out_names,
        "zero_shapes": zero_shapes,
        "n_params": n_params,
    }
    _CACHE["runner"] = runner
    return runner


def _prep_in_maps(inputs):
    f16 = np.float16

    x = np.ascontiguousarray(inputs["x"], dtype=np.float32)
    y = np.ascontiguousarray(inputs["y"], dtype=np.float32)

    Ax, cx = _fold(inputs["Wx1"], inputs["sx1"], inputs["bx1"],
                   inputs["Wx2"], inputs["sx2"], inputs["bx2"])
    As, cs = _fold(inputs["Ws1"], inputs["ss1"], inputs["bs1"],
                   inputs["Ws2"], inputs["ss2"], inputs["bs2"])
    Ay, cy = _fold(inputs["Wy1"], inputs["sy1"], inputs["by1"],
                   inputs["Wy2"], inputs["sy2"], inputs["by2"])
    Au = inputs["su"].astype(np.float64)[:, None] * inputs["Wu"].astype(np.float64)
    cu = inputs["bu"].astype(np.float64)

    # [768, 128] proj weights, zero-padded rows 720:768
    wks = np.zeros((CXP, 128), np.float32)
    wks[0:CX] = np.concatenate([Ax.T, As.T], axis=1)
    wks = wks.astype(f16)
    # packed y weights: block-diag [[Ay.T, 0], [0, Ay.T]]
    wyd = np.zeros((128, 128), np.float32)
    wyd[0:CM, 0:CM] = Ay.T
    wyd[CM:128, CM:128] = Ay.T
    wyd = wyd.astype(f16)
    # packed up weights: per 128-chunk of (padded) output channels,
    # block-diag of two 64-wide halves
    AuTp = np.zeros((CM, CXP), np.float32)
    AuTp[:, 0:CX] = Au.T.astype(np.float32)
    wud = np.zeros((128, CXP), np.float32)
    for j in range(KC):
        wud[0:CM, j * 128:j * 128 + CM] = AuTp[:, j * 128:j * 128 + CM]
        wud[CM:128, j * 128 + CM:(j + 1) * 128] = \
            AuTp[:, j * 128 + CM:(j + 1) * 128]
    wud = wud.astype(f16)

    bxs = np.concatenate([cx, cs])[:, None].astype(np.float32)  # [128, 1]
    by = cy[:, None].astype(np.float32)
    bu = np.zeros((CXP, 1), np.float32)
    bu[0:CX] = cu[:, None].astype(np.float32)

    in_maps = []
    for c in range(NCORES):
        b, r = divmod(c, 4)
        xq = np.zeros((CXP, NL), f16)
        xq[0:CX] = x[b, :, r * RB:(r + 1) * RB, :].reshape(CX, NL).astype(f16)
        yq = np.ascontiguousarray(
            y[b, :, r * RB:(r + 1) * RB, :].reshape(CM, NL)).astype(f16)
        in_maps.append({
            "xq": xq, "yq": yq, "wks": wks, "wyd": wyd, "wud": wud,
            "bxs": bxs, "by": by, "bu": bu,
        })
    return in_maps


def _run(in_maps):
    r = _get_runner()
    concat_in = [
        np.concatenate([in_maps[c][name] for c in range(NCORES)], axis=0)
        for name in r["in_names"]
    ]
    if "dz" not in _CACHE:
        import jax
        from jax.sharding import Mesh, PartitionSpec, NamedSharding
        mesh = Mesh(np.asarray(jax.devices()[:NCORES]), ("core",))
        sh = NamedSharding(mesh, PartitionSpec("core"))
        _CACHE["dz"] = [
            jax.device_put(np.zeros((NCORES * s[0], *s[1:]), dt), sh)
            for (s, dt) in r["zero_shapes"]
        ]
    out_arrs = r["sharded"](*concat_in, *_CACHE["dz"])
    outs = []
    for i, name in enumerate(r["out_names"]):
        arr = np.asarray(out_arrs[i])
        outs.append(arr.reshape(NCORES, -1, arr.shape[-1]))
    return {name: outs[i] for i, name in enumerate(r["out_names"])}


def kernel(**inputs):
    in_maps = _prep_in_maps(inputs)
    res = _run(in_maps)
    o = res["out"]  # [8, 768, 4096] fp16 (rows 720:768 are pad)
    out = np.empty((B, CX, H, W), dtype=np.float32)
    for c in range(NCORES):
        b, r = divmod(c, 4)
        out[b, :, r * RB:(r + 1) * RB, :] = (
            o[c][0:CX].astype(np.float32).reshape(CX, RB, W))
    return out


# revision 26
# speedup vs baseline: 1.0574x; 1.0574x over previous
"""Trainium2 Bass kernel for nn_BCAModule (bilateral cross-attention).

Full inputs in, full outputs out. Internally sharded over 8 NeuronCores:
core c handles batch b = c // 4 and query rows 32*(c%4) .. 32*(c%4)+32
(N_loc = 4096 of the N = 16384 queries). Pooled K/V ([64, 1024]) is built
cooperatively: each core pools its own spatial quarter ([64, 256]) and the
4-core group all-gathers.

All heavy tensors ride in fp16 (inputs cast host-side, matmuls fp16 with
f32 PSUM accumulation, output written fp16 and upcast host-side). sim is
in [-9, 8] for these inputs so exp(sim) fits fp16 range comfortably.

Every hot matmul runs with a full K=128 contraction: the PE clock gate
(HAM) only releases the 2.4 GHz clock when array utilization is high, and
half-height (K=64) matmuls run at 1.2 GHz forever. So:
  - the input projection is padded host-side to 768 channels (6x K=128),
  - the y projection packs two n-tiles via a block-diagonal [[Ay],[Ay]],
  - sim packs two 64-wide m chunks via block-diagonal fy tiles against a
    duplicated-fx rhs,
  - the up-projection packs two 60-channel output chunks via a
    block-diagonal wu (built host-side) against a duplicated-fout rhs.

Pipelining: pools are hoisted out of the repeat loop (per-tag buffer
rotation); the repeat loop is software-pipelined so each n-tile
interleaves attention of iteration k-1 with projections of iteration k,
keeping the PE stream dense. Loads issue on the sync(SP) HWDGE ring,
stores on the scalar(ACT) ring, gather-unpack on gpsimd SWDGE.
"""

import numpy as np

B, CX, CM, H, W = 2, 720, 64, 128, 128
CXP = 768             # padded input channels (6 x 128)
NCORES = 8
RB = 32               # image rows per core
NL = RB * W           # 4096 local queries
ML = (RB // 4) * (W // 4)   # 256 local pooled positions
M = 4 * ML            # 1024 pooled positions per batch
KC = CXP // 128       # 6 proj contraction chunks of 128
KPO = 120             # output-channel chunk (two 60-halves per packed MM)
OC = CX // KPO        # 6
NT = 512              # n tile
NTN = NL // NT        # 8
MCH = 128             # m chunk
NMC = M // MCH        # 8
GR = 2 * ML           # gather rows per core: fs^T block + fy block

_CACHE = {}


def _build_nc(repeat=1, phases=(1, 2)):
    import concourse.bass as bass
    from concourse import bacc
    import concourse.mybir as mybir
    import concourse.tile as tile
    from concourse.masks import make_identity

    F32 = mybir.dt.float32
    F16 = mybir.dt.float16
    AF = mybir.ActivationFunctionType
    ALU = mybir.AluOpType

    nc = bacc.Bacc(None)

    xq_d = nc.dram_tensor("xq", [CXP, NL], F16, kind="ExternalInput")
    yq_d = nc.dram_tensor("yq", [CM, NL], F16, kind="ExternalInput")
    wks_d = nc.dram_tensor("wks", [CXP, 128], F16, kind="ExternalInput")
    wyd_d = nc.dram_tensor("wyd", [128, 128], F16, kind="ExternalInput")
    wud_d = nc.dram_tensor("wud", [128, CXP], F16, kind="ExternalInput")
    bxs_d = nc.dram_tensor("bxs", [128, 1], F32, kind="ExternalInput")
    by_d = nc.dram_tensor("by", [CM, 1], F32, kind="ExternalInput")
    bu_d = nc.dram_tensor("bu", [CXP, 1], F32, kind="ExternalInput")
    out_d = nc.dram_tensor("out", [CXP, NL], F16, kind="ExternalOutput")

    with tile.TileContext(nc) as tc:
        with (
            tc.tile_pool(name="wpool", bufs=1) as wp,
            tc.tile_pool(name="xpool", bufs=1) as xp,
            tc.tile_pool(name="persist", bufs=1) as pers,
            tc.tile_pool(name="p1sb", bufs=1) as p1,
            tc.tile_pool(name="p2sb", bufs=1) as p2,
            tc.tile_pool(name="pjps", bufs=1, space="PSUM") as pj,
            tc.tile_pool(name="p2ps", bufs=1, space="PSUM") as p2p,
            tc.tile_pool(name="dram", bufs=2, space="DRAM") as dp,
        ):
            # ---------------- weights / constants (once) ----------------
            w_ks_t = wp.tile([128, KC * 128], F16, tag="wks")
            nc.sync.dma_start(
                w_ks_t[:].rearrange("p (k m) -> p k m", k=KC),
                wks_d[:].rearrange("(k p) m -> p k m", k=KC),
            )
            w_ks = w_ks_t[:]

            w_yd_t = wp.tile([128, 128], F16, tag="wyd")
            nc.sync.dma_start(w_yd_t[:], wyd_d[:])
            w_yd = w_yd_t[:]

            w_ud_t = wp.tile([128, CXP], F16, tag="wud")
            nc.sync.dma_start(w_ud_t[:], wud_d[:])
            w_ud = w_ud_t[:]

            bxs_sb = wp.tile([128, 1], F32, tag="bxs")
            nc.sync.dma_start(bxs_sb[:], bxs_d[:])
            by_sb = wp.tile([CM, 1], F32, tag="by")
            nc.sync.dma_start(by_sb[:], by_d[:])
            bu_sb = wp.tile([128, KC], F32, tag="bu")
            nc.sync.dma_start(
                bu_sb[:].rearrange("p (k o) -> p k o", k=KC),
                bu_d[:].rearrange("(k p) o -> p k o", k=KC),
            )

            ident = wp.tile([128, 128], F16, tag="ident")
            make_identity(nc, ident[:])
            ones_f = wp.tile([128, 1], F16, tag="ones_f")
            nc.gpsimd.memset(ones_f[:], 1.0)
            ones_row = wp.tile([1, CM], F32, tag="ones_row")
            nc.gpsimd.memset(ones_row[:], 1.0)


            def phase1_head(it):
                st = {}
                st["g_in"] = dp.tile([GR, CM], F16, tag="g_in", name="g_in")
                st["g_out"] = dp.tile([4 * GR, CM], F16, tag="g_out",
                                      name="g_out")
                st["fxd"] = pers.tile([128, NL], F16, tag="fxd", bufs=2,
                                      name="fxd")
                # block-diagonal fy tiles [[fy_a, 0], [0, fy_b]],
                # double-buffered across reps; re-zeroed each rep
                st["fyd"] = [
                    pers.tile([128, 128], F16, tag=f"fyd{mc}", bufs=2,
                              name=f"fyd{mc}")
                    for mc in range(NMC)
                ]
                for mc in range(NMC):
                    nc.gpsimd.memset(st["fyd"][mc][:], 0.0)
                st["fsa"] = [
                    pers.tile([MCH, 65], F16, tag=f"fsa{mc}", bufs=2,
                              name=f"fsa{mc}")
                    for mc in range(NMC)
                ]
                st["x"] = [
                    xp.tile([128, NL], F16, tag=f"x{k}", bufs=2, name=f"x{k}")
                    for k in range(KC)
                ]
                for k in range(KC):
                    nc.sync.dma_start(
                        st["x"][k][:], xq_d[k * 128:(k + 1) * 128, :])
                # y in packed layout: rows 0:64 = first half cols,
                # rows 64:128 = second half cols
                st["y2"] = p1.tile([128, NL // 2], F16, tag="y2", bufs=2,
                                   name="y2")
                nc.sync.dma_start(st["y2"][0:CM, :], yq_d[:, 0:NL // 2])
                nc.sync.dma_start(st["y2"][CM:128, :], yq_d[:, NL // 2:NL])
                st["fy_p1"] = p1.tile([CM, RB * 32], F32, tag="pool_p1",
                                      bufs=2, name="fy_p1")
                st["fs_p1"] = p1.tile([CM, RB * 32], F16, tag="fs_p1", bufs=2,
                                      name="fs_p1")
                return st

            def proj_nt(st, nt):
                ns = slice(nt * NT, (nt + 1) * NT)
                if nt < NTN // 2:
                    # packed y projection: one K=128 MM covers n-tiles
                    # nt and nt + 4
                    ps = pj.tile([128, NT], F32, tag="pp", bufs=1, name="ps_y")
                    nc.tensor.matmul(
                        ps[:], w_yd, st["y2"][:, ns], start=True, stop=True)
                    nc.vector.tensor_reduce(
                        st["fy_p1"][:, nt * 128:(nt + 1) * 128],
                        ps[0:CM, :].rearrange("p (a w) -> p a w", w=4),
                        axis=mybir.AxisListType.X, op=ALU.max,
                    )
                    nc.vector.tensor_reduce(
                        st["fy_p1"][:, (nt + 4) * 128:(nt + 5) * 128],
                        ps[CM:128, :].rearrange("p (a w) -> p a w", w=4),
                        axis=mybir.AxisListType.X, op=ALU.max,
                    )
                pp = pj.tile([128, NT], F32, tag="pp", bufs=1, name="pp")
                for k in range(KC):
                    nc.tensor.matmul(
                        pp[:],
                        w_ks[:, k * 128:(k + 1) * 128],
                        st["x"][k][:, ns],
                        start=(k == 0),
                        stop=(k == KC - 1),
                    )
                # fx duplicated to both partition halves (sim rhs);
                # fself pooled straight off PSUM (bias folded post-pool)
                nc.vector.tensor_scalar_add(
                    st["fxd"][0:CM, ns], pp[0:CM, :], bxs_sb[0:CM, :])
                nc.vector.tensor_scalar_add(
                    st["fxd"][CM:128, ns], pp[0:CM, :], bxs_sb[0:CM, :])
                nc.vector.tensor_reduce(
                    st["fs_p1"][:, nt * 128:(nt + 1) * 128],
                    pp[CM:128, :].rearrange("p (a w) -> p a w", w=4),
                    axis=mybir.AxisListType.X, op=ALU.max,
                )

            def phase1_tail(st):
                fy_pool = p1.tile([CM, ML], F32, tag="fy_pool", bufs=2)
                nc.vector.tensor_reduce(
                    fy_pool[:],
                    st["fy_p1"][:].rearrange(
                        "p (hb hh wb) -> p hb wb hh", hb=RB // 4, hh=4),
                    axis=mybir.AxisListType.X, op=ALU.max,
                )
                fy_poolb = p1.tile([CM, ML], F16, tag="fy_poolb", bufs=2)
                nc.vector.tensor_scalar_add(fy_poolb[:], fy_pool[:], by_sb[:])
                fs_pool = p1.tile([CM, ML], F16, tag="fs_pool", bufs=2)
                nc.vector.tensor_reduce(
                    fs_pool[:],
                    st["fs_p1"][:].rearrange(
                        "p (hb hh wb) -> p hb wb hh", hb=RB // 4, hh=4),
                    axis=mybir.AxisListType.X, op=ALU.max,
                )
                fs_poolb = p1.tile([CM, ML], F16, tag="fs_poolb", bufs=2)
                nc.vector.tensor_scalar_add(
                    fs_poolb[:], fs_pool[:], bxs_sb[CM:128, :])
                # gather block: rows 0:256 = fs^T, rows 256:512 = fy
                fst = p1.tile([128, 128], F16, tag="fst", bufs=2)
                for j in range(2):
                    tps = p2p.tile([128, CM], F16, tag="up", bufs=2, name="tps")
                    nc.tensor.transpose(
                        tps[:], fs_poolb[:, j * 128:(j + 1) * 128],
                        ident[0:CM, 0:CM],
                    )
                    nc.vector.tensor_copy(fst[:, j * CM:(j + 1) * CM], tps[:])
                nc.scalar.dma_start(
                    st["g_in"][0:ML, :].rearrange("(j p) c -> p j c", p=128),
                    fst[:].rearrange("p (j c) -> p j c", j=2),
                )
                nc.scalar.dma_start(
                    st["g_in"][ML:GR, :].rearrange("(c a) b -> c (a b)", a=4),
                    fy_poolb[:],
                )
                if "nocc" not in phases:
                    nc.gpsimd.collective_compute(
                        "AllGather",
                        ALU.bypass,
                        replica_groups=[[0, 1, 2, 3], [4, 5, 6, 7]],
                        ins=[st["g_in"][:].opt()],
                        outs=[st["g_out"][:].opt()],
                    )
                # unpack gathered K/V: fsa direct, fy into block-diag tiles
                for r in range(4):
                    base = r * GR
                    for h in range(2):
                        mc = 2 * r + h
                        nc.gpsimd.dma_start(
                            st["fsa"][mc][:, 0:CM],
                            st["g_out"][base + h * 128:base + (h + 1) * 128, :],
                        )
                        nc.vector.tensor_copy(
                            st["fsa"][mc][:, CM:65], ones_f[:])
                    fy_rows = st["g_out"][base + ML:base + GR, :].rearrange(
                        "(c a) b -> a c b", a=4)
                    for a in range(4):
                        mc = 2 * r + a // 2
                        h = a % 2
                        nc.gpsimd.dma_start(
                            st["fyd"][mc][h * CM:(h + 1) * CM,
                                          h * CM:(h + 1) * CM],
                            fy_rows[a],
                        )

            def attn_sim(st, nt):
                ns = slice(nt * NT, (nt + 1) * NT)
                et = p2.tile([128, NMC * NT], F16, tag="et", bufs=2)
                for mh in range(NMC // 2):
                    st2 = p2p.tile([128, 2 * NT], F32, tag="sim", bufs=2,
                                   name="st2")
                    nc.tensor.matmul(
                        st2[:, 0:NT], st["fyd"][2 * mh][:], st["fxd"][:, ns],
                        start=True, stop=True,
                    )
                    nc.tensor.matmul(
                        st2[:, NT:2 * NT], st["fyd"][2 * mh + 1][:],
                        st["fxd"][:, ns], start=True, stop=True,
                    )
                    nc.scalar.activation(
                        et[:, 2 * mh * NT:(2 * mh + 2) * NT], st2[:], AF.Exp,
                    )
                return et

            def attn_tail(st, nt, et):
                ns = slice(nt * NT, (nt + 1) * NT)
                fo = p2p.tile([65, NT], F32, tag="fo", bufs=1)
                for mc in range(NMC):
                    nc.tensor.matmul(
                        fo[:], st["fsa"][mc][:], et[:, mc * NT:(mc + 1) * NT],
                        start=(mc == 0), stop=(mc == NMC - 1),
                    )
                zrow = p2.tile([1, NT], F32, tag="zrow", bufs=2)
                nc.vector.tensor_copy(zrow[:], fo[CM:65, :])
                rz = p2.tile([1, NT], F32, tag="rz", bufs=2)
                nc.vector.reciprocal_approx_fast(rz[:], zrow[:])
                rzb_ps = p2p.tile([128, NT], F32, tag="sim", bufs=2, name="rzb")
                nc.tensor.matmul(
                    rzb_ps[0:CM, :], ones_row[:], rz[:], start=True, stop=True)
                rzb_sb = p2.tile([CM, NT], F32, tag="rzb_sb", bufs=2)
                nc.vector.tensor_copy(rzb_sb[:], rzb_ps[0:CM, :])
                # fout duplicated to both halves (packed up-proj rhs)
                fout2 = p2.tile([128, NT], F16, tag="fout2", bufs=2)
                nc.vector.tensor_tensor(
                    fout2[0:CM, :], fo[0:CM, :], rzb_sb[:], op=ALU.mult)
                nc.vector.tensor_tensor(
                    fout2[CM:128, :], fo[0:CM, :], rzb_sb[:], op=ALU.mult)
                ob = p2.tile([128, KC * NT], F16, tag="ob", bufs=2)
                for ot in range(KC):
                    xr = p2.tile([128, NT], F16, tag="xr", bufs=6)
                    nc.sync.dma_start(xr[:], xq_d[ot * 128:(ot + 1) * 128, ns])
                    up = p2p.tile([128, NT], F32, tag="up", bufs=2, name="up")
                    nc.tensor.matmul(
                        up[:], w_ud[:, ot * 128:(ot + 1) * 128],
                        fout2[:], start=True, stop=True,
                    )
                    nc.vector.scalar_tensor_tensor(
                        ob[:, ot * NT:(ot + 1) * NT], up[:],
                        bu_sb[:, ot:ot + 1], xr[:],
                        op0=ALU.add, op1=ALU.add,
                    )
                nc.scalar.dma_start(
                    out_d[:, ns].rearrange("(o p) n -> p o n", o=KC),
                    ob[:].rearrange("p (o n) -> p o n", o=KC),
                )

            # software pipeline: iteration `it` runs phase 1 of rep `it`
            # interleaved (per n-tile) with the attention of rep `it-1`,
            # so projection matmuls fill the PE bubbles left by the
            # exp/normalize chains and keep the HAM clock warm.
            prev = None
            for it in range(repeat + 1):
                cur = phase1_head(it) if it < repeat else None
                for nt in range(NTN):
                    et = attn_sim(prev, nt) if prev is not None else None
                    if cur is not None:
                        proj_nt(cur, nt)
                    if prev is not None:
                        attn_tail(prev, nt, et)
                if cur is not None:
                    phase1_tail(cur)
                prev = cur

    nc.finalize()
    return nc


def _fold(W1, s1, b1, W2, s2, b2):
    W1 = W1.astype(np.float64)
    W2 = W2.astype(np.float64)
    A1 = s1.astype(np.float64)[:, None] * W1
    A2 = s2.astype(np.float64)[:, None] * W2
    A = A2 @ A1
    c = A2 @ b1.astype(np.float64) + b2.astype(np.float64)
    return A, c


def _get_runner():
    if "runner" in _CACHE:
        return _CACHE["runner"]

    import jax
    import concourse.mybir as mybir
    from jax.sharding import Mesh, PartitionSpec
    from jax.experimental.shard_map import shard_map
    from concourse.bass2jax import (
        _bass_exec_p, install_neuronx_cc_hook, partition_id_tensor,
    )

    nc = _build_nc()
    install_neuronx_cc_hook()

    partition_name = nc.partition_id_tensor.name if nc.partition_id_tensor else None
    in_names, out_names, out_avals, zero_shapes = [], [], [], []
    for alloc in nc.m.functions[0].allocations:
        if not isinstance(alloc, mybir.MemoryLocationSet):
            continue
        if getattr(alloc, "kind", None) == "ExternalInput":
            name = alloc.memorylocations[0].name
            if name != partition_name:
                in_names.append(name)
        elif getattr(alloc, "kind", None) == "ExternalOutput":
            name = alloc.memorylocations[0].name
            out_names.append(name)
            shape = tuple(alloc.tensor_shape)
            dtype = mybir.dt.np(alloc.dtype)
            out_avals.append(jax.core.ShapedArray(shape, dtype))
            zero_shapes.append((shape, dtype))

    n_params = len(in_names)
    n_outs = len(out_avals)
    all_in_names = list(in_names) + list(out_names)
    if partition_name is not None:
        all_in_names.append(partition_name)

    def _body(*args):
        operands = list(args)
        if partition_name is not None:
            operands.append(partition_id_tensor())
        outs = _bass_exec_p.bind(
            *operands,
            out_avals=tuple(out_avals),
            in_names=tuple(all_in_names),
            out_names=tuple(out_names),
            lowering_input_output_aliases=(),
            sim_require_finite=True,
            sim_require_nnan=True,
            nc=nc,
        )
        return tuple(outs)

    devices = jax.devices()[:NCORES]
    mesh = Mesh(np.asarray(devices), ("core",))
    in_specs = (PartitionSpec("core"),) * (n_params + n_outs)
    out_specs = (PartitionSpec("core"),) * n_outs
    sharded = jax.jit(
        shard_map(_body, mesh=mesh, in_specs=in_specs, out_specs=out_specs,
                  check_rep=False),
        keep_unused=True,
    )

    runner = {
        "sharded": sharded,
        "in_names": in_names,
        "out_names": out_names,
        "zero_shapes": zero_shapes,
        "n_params": n_params,
    }
    _CACHE["runner"] = runner
    return runner


def _prep_in_maps(inputs):
    f16 = np.float16

    x = np.ascontiguousarray(inputs["x"], dtype=np.float32)
    y = np.ascontiguousarray(inputs["y"], dtype=np.float32)

    Ax, cx = _fold(inputs["Wx1"], inputs["sx1"], inputs["bx1"],
                   inputs["Wx2"], inputs["sx2"], inputs["bx2"])
    As, cs = _fold(inputs["Ws1"], inputs["ss1"], inputs["bs1"],
                   inputs["Ws2"], inputs["ss2"], inputs["bs2"])
    Ay, cy = _fold(inputs["Wy1"], inputs["sy1"], inputs["by1"],
                   inputs["Wy2"], inputs["sy2"], inputs["by2"])
    Au = inputs["su"].astype(np.float64)[:, None] * inputs["Wu"].astype(np.float64)
    cu = inputs["bu"].astype(np.float64)

    # [768, 128] proj weights, zero-padded rows 720:768
    wks = np.zeros((CXP, 128), np.float32)
    wks[0:CX] = np.concatenate([Ax.T, As.T], axis=1)
    wks = wks.astype(f16)
    # packed y weights: block-diag [[Ay.T, 0], [0, Ay.T]]
    wyd = np.zeros((128, 128), np.float32)
    wyd[0:CM, 0:CM] = Ay.T
    wyd[CM:128, CM:128] = Ay.T
    wyd = wyd.astype(f16)
    # packed up weights: per 128-chunk of (padded) output channels,
    # block-diag of two 64-wide halves
    AuTp = np.zeros((CM, CXP), np.float32)
    AuTp[:, 0:CX] = Au.T.astype(np.float32)
    wud = np.zeros((128, CXP), np.float32)
    for j in range(KC):
        wud[0:CM, j * 128:j * 128 + CM] = AuTp[:, j * 128:j * 128 + CM]
        wud[CM:128, j * 128 + CM:(j + 1) * 128] = \
            AuTp[:, j * 128 + CM:(j + 1) * 128]
    wud = wud.astype(f16)

    bxs = np.concatenate([cx, cs])[:, None].astype(np.float32)  # [128, 1]
    by = cy[:, None].astype(np.float32)
    bu = np.zeros((CXP, 1), np.float32)
    bu[0:CX] = cu[:, None].astype(np.float32)

    in_maps = []
    for c in range(NCORES):
        b, r = divmod(c, 4)
        xq = np.zeros((CXP, NL), f16)
        xq[0:CX] = x[b, :, r * RB:(r + 1) * RB, :].reshape(CX, NL).astype(f16)
        yq = np.ascontiguousarray(
            y[b, :, r * RB:(r + 1) * RB, :].reshape(CM, NL)).astype(f16)
        in_maps.append({
            "xq": xq, "yq": yq, "wks": wks, "wyd": wyd, "wud": wud,
            "bxs": bxs, "by": by, "bu": bu,
        })
    return in_maps


def _run(in_maps):
    r = _get_runner()
    concat_in = [
        np.concatenate([in_maps[c][name] for c in range(NCORES)], axis=0)
        for name in r["in_names"]
    ]
    if "dz" not in _CACHE:
        import jax
        from jax.sharding import Mesh, PartitionSpec, NamedSharding
        mesh = Mesh(np.asarray(jax.devices()[:NCORES]), ("core",))
        sh = NamedSharding(mesh, PartitionSpec("core"))
        _CACHE["dz"] = [
            jax.device_put(np.zeros((NCORES * s[0], *s[1:]), dt), sh)
            for (s, dt) in r["zero_shapes"]
        ]
    out_arrs = r["sharded"](*concat_in, *_CACHE["dz"])
    outs = []
    for i, name in enumerate(r["out_names"]):
        arr = np.asarray(out_arrs[i])
        outs.append(arr.reshape(NCORES, -1, arr.shape[-1]))
    return {name: outs[i] for i, name in enumerate(r["out_names"])}


def kernel(**inputs):
    in_maps = _prep_in_maps(inputs)
    res = _run(in_maps)
    o = res["out"]  # [8, 768, 4096] fp16 (rows 720:768 are pad)
    out = np.empty((B, CX, H, W), dtype=np.float32)
    for c in range(NCORES):
        b, r = divmod(c, 4)
        out[b, :, r * RB:(r + 1) * RB, :] = (
            o[c][0:CX].astype(np.float32).reshape(CX, RB, W))
    return out
